# revision 1
# baseline (speedup 1.0000x reference)
"""AttentionOCR spatial self-attention kernel for Trainium2 (Bass/Tile).

Reference computation (per batch element b):
    q = w1 @ x + b1           [32, N]    (used transposed: [N, 32])
    k = w2 @ x + b2           [32, N]
    v = w3 @ x + b3           [256, N]
    en[i, j] = q[:, i] . k[:, j]
    attn = softmax_j(en)
    out = gamma * (v @ attn^T) + x

Sharding: 8 cores = 4 batches x 2 row-halves. Core (b, h) receives x[b]
ROTATED left by h*2048 columns, so its 2048 query rows are always columns
0:2048 of its local x. Attention is permutation-invariant in j (softmax
sums over all keys), so k/v built from the rotated x give identical
results; the host re-assembles the output halves.

Layout / performance choices (all measured on hw):
  - scores are computed TRANSPOSED (enT[j, i]): after exp, the probability
    tile [j-part, i-free] is directly the moving operand of the PV matmul.
  - softmax row sums come from an all-ones stationary matmul accumulated
    over j-chunks in PSUM; max-subtraction is skipped (|en| <= ~30).
  - v's bias b3 is folded into the finalize (attn rows sum to 1):
      out = gamma * (pv * (1/s) + b3) + x
  - all matmuls run in float32r (1 PE cycle/row vs 4 for float32 when the
    moving free size is >= 256). f32r operands must come from a rounding
    producer, so x/weights get one-time converted copies and q/k/exp(en)/vT
    are written as f32r by the activation or copy that produces them anyway.
  - q/k (and w1/w2/b1/b2) are zero-padded from 32 to 128 rows so every
    matmul is a uniform (128,128) PE tile: a matmul right after a PE
    tile-size switch runs at HALF rate (430 vs 231 ns at F=512).
  - the en matmul for j-chunk jc+L is issued ahead of the accumulation
    matmuls of jc (L=3 in steady state), hiding the scalar-engine EXP
    latency so the in-order PE queue never stalls.
  - x takes ~12 us to land (4 MiB at HBM bandwidth), so attention i-block
    0 is fused with the projection phase: per 512-col block, project
    q/k/vT, then run the attention group of the previous block's j-chunks.
  - softmax normalization uses reciprocal_approx_fast (~18 bits, 5x
    faster than reciprocal); finalize tails run on the otherwise-idle
    GpSimd except the exposed last i-block.
"""

import numpy as np

import concourse.bass as bass
import concourse.mybir as mybir
import concourse.tile as tile
from concourse import bacc, bass_utils
from concourse.bass import ts

F32 = mybir.dt.float32
F32R = mybir.dt.float32r
AF = mybir.ActivationFunctionType
OP = mybir.AluOpType

B, C, H, W = 4, 256, 64, 64
N = H * W              # 4096 spatial positions
CQK = C // 8           # 32
NCORES = 8
HALF = N // 2          # 2048 rows of attention per core
P = 128
KO = C // P            # 2 contraction chunks of 128
NJ = N // P            # 32 j-chunks
IBLK = 512             # i-block (columns of enT) per inner pass
NIB = HALF // IBLK     # 4

_cache = {}
last_results = None    # BassKernelResults of the most recent run (for test.py)


def _build_nc(bench_iters=0):
    nc = bacc.Bacc("TRN2", debug=False, num_devices=NCORES)

    x = nc.dram_tensor("x", [C, N], F32, kind="ExternalInput").ap()
    w1t = nc.dram_tensor("w1t", [C, CQK], F32, kind="ExternalInput").ap()
    w2t = nc.dram_tensor("w2t", [C, CQK], F32, kind="ExternalInput").ap()
    w3t = nc.dram_tensor("w3t", [C, C], F32, kind="ExternalInput").ap()
    b1 = nc.dram_tensor("b1", [CQK], F32, kind="ExternalInput").ap()
    b2 = nc.dram_tensor("b2", [CQK], F32, kind="ExternalInput").ap()
    b3 = nc.dram_tensor("b3", [C], F32, kind="ExternalInput").ap()
    gamma = nc.dram_tensor("gamma", [1], F32, kind="ExternalInput").ap()
    out = nc.dram_tensor("out", [C, HALF], F32, kind="ExternalOutput").ap()

    with tile.TileContext(nc) as tc:
        _emit(tc, out, x, w1t, w2t, w3t, b1, b2, b3, gamma,
              bench_iters=bench_iters)
    nc.compile()
    return nc


def _emit(tc, out, x, w1t, w2t, w3t, b1, b2, b3, gamma, bench_iters=0):
    nc = tc.nc
    from contextlib import ExitStack

    with ExitStack() as ctx:
        if bench_iters:
            ctx.enter_context(tc.For_i(0, bench_iters, 1))
        consts = ctx.enter_context(tc.tile_pool(name="consts", bufs=1))

        # ---- startup: ordered for earliest possible first projection ---
        # Issue queues in parallel: Sync carries gamma + the x chunks
        # (<=1 MiB each, so no single DMA queue serializes), Scalar-HWDGE
        # carries the weights/biases. Each DIRECT2D issue costs ~640 ns of
        # sequencer time, so order within a queue = priority.
        # w1/w2 (and b1/b2) are zero-padded from 32 to 128 output channels
        # so q/k live on 128 partitions and EVERY matmul in the kernel is a
        # uniform (128,128) PE tile — a matmul right after a PE tile-size
        # switch runs at half rate (measured 430 vs 231 ns for F=512).
        x_sb = consts.tile([P, KO, N], F32)
        x_r = consts.tile([P, KO, N], F32R)
        xr = x.rearrange("(ko ki) n -> ki ko n", ki=P)
        w1t_sb = consts.tile([P, KO, P], F32)
        w2t_sb = consts.tile([P, KO, P], F32)
        w3t_sb = consts.tile([P, KO, C], F32)
        b1_sb = consts.tile([P, 1], F32)
        b2_sb = consts.tile([P, 1], F32)
        b3_sb = consts.tile([P, KO], F32)
        g1_sb = consts.tile([1, 1], F32)

        nc.vector.memset(w1t_sb, 0.0)
        nc.vector.memset(w2t_sb, 0.0)
        nc.vector.memset(b1_sb, 0.0)
        nc.vector.memset(b2_sb, 0.0)

        # x in 8x512-col chunks (512 KiB each, ~93 GB/s per DMA queue),
        # issued alternately from the Sync and Scalar HWDGE queues so the
        # ~640ns-per-issue sequencer cost is paid in parallel.
        XB = 512
        NXB = N // XB  # 8
        # weights/biases on the Scalar HWDGE queue; x on Sync, with the
        # chunk DMAs CHAINED head-to-tail: concurrent DMAs fair-share HBM
        # bandwidth (everything lands late, ~24us); serialized, chunk b
        # lands at ~13+5.5b us, just ahead of when the fused projection/
        # attention phase consumes it.
        nc.scalar.dma_start(g1_sb, gamma[None, :])
        # w3 first: it is 256 KB of scattered rows (descriptor-bound, ~3us
        # to land) and gates the v projections at ~13.5us — issued late it
        # left a measured 2.7us PE stall.
        nc.scalar.dma_start(w3t_sb,
                            w3t.rearrange("(ko ki) m -> ki ko m", ki=P))
        nc.scalar.dma_start(w1t_sb[:, :, 0:CQK],
                            w1t.rearrange("(ko ki) m -> ki ko m", ki=P))
        nc.scalar.dma_start(b1_sb[0:CQK, :], b1[:, None])
        nc.scalar.dma_start(w2t_sb[:, :, 0:CQK],
                            w2t.rearrange("(ko ki) m -> ki ko m", ki=P))
        for b in range(NXB):
            d = nc.sync.dma_start(x_sb[:, :, ts(b, XB)], xr[:, :, ts(b, XB)])
            tc.chain_iter_dep("xchain", d.ins)
        nc.scalar.dma_start(b2_sb[0:CQK, :], b2[:, None])
        nc.scalar.dma_start(b3_sb, b3.rearrange("(ko ki) -> ki ko", ki=P))

        # f32r rounding copies (the verifier requires a rounding producer
        # for f32r matmul operands) — all on Vector, in landing order, so
        # the Scalar queue stays free for the projection activations.
        ones_f32 = consts.tile([P, P], F32)
        nc.vector.memset(ones_f32, 1.0)
        ones_sb = consts.tile([P, P], F32R)
        nc.gpsimd.tensor_copy(ones_sb, ones_f32)
        w1t_r = consts.tile([P, KO, P], F32R)
        w2t_r = consts.tile([P, KO, P], F32R)
        w3t_r = consts.tile([P, KO, C], F32R)
        nc.vector.tensor_copy(w1t_r, w1t_sb)

        def conv_chunk(b):
            for ko in range(KO):
                nc.vector.tensor_copy(x_r[:, ko, ts(b, XB)],
                                      x_sb[:, ko, ts(b, XB)])

        conv_chunk(0)
        nc.vector.tensor_copy(w2t_r, w2t_sb)
        nc.vector.tensor_copy(w3t_r, w3t_sb)

        # gamma broadcast to all 128 partitions via a K=1 matmul
        # (avoids a partition-stride-0 DMA, which only SWDGE supports).
        ones1_f32 = consts.tile([1, P], F32)
        nc.vector.memset(ones1_f32, 1.0)
        gamma_sb = consts.tile([P, 1], F32)
        with tc.tile_pool(name="g_ps", bufs=1, space="PSUM") as gps:
            gp = gps.tile([P, 1], F32)
            nc.tensor.matmul(gp, ones1_f32, g1_sb, start=True, stop=True)
            nc.vector.tensor_copy(gamma_sb, gp)
        g3_sb = consts.tile([P, KO], F32)  # b3 * gamma (finalize bias)
        nc.vector.tensor_scalar(g3_sb, b3_sb, gamma_sb, None, OP.mult)

        qsb = consts.tile([P, HALF], F32R)
        ksb = consts.tile([P, N], F32R)
        vts = consts.tile([P, NJ, C], F32R)

        PB = 512
        outr = out.rearrange("(ko ki) n -> ki ko n", ki=P)

        def finalize(ib, pv0, pv1, sp, fin, last_ib):
            # finalize: out = gamma * (pv/s + b3) + x   (reads PSUM
            # directly; split across engines so the banks free quickly)
            rs = fin.tile([P, IBLK], F32, tag="rs")
            # ~18 correct bits, 5x faster than reciprocal(); s is a sum
            # of positive exps (no zeros/denorms/infs possible)
            nc.vector.reciprocal_approx_fast(rs, sp)
            ot = fin.tile([P, KO, IBLK], F32, tag="ot")
            for cc, pv in enumerate((pv0, pv1)):
                # PSUM reads must be on Vector. The t2/ot tail runs on
                # the otherwise-idle GpSimd so Scalar keeps feeding EXP;
                # on the last i-block (exposed tail, latency matters)
                # t2 goes to Scalar and ot to Vector instead.
                t = fin.tile([P, IBLK], F32, tag=f"t{cc}")
                nc.vector.tensor_tensor(t, pv, rs, OP.mult)
                t2 = fin.tile([P, IBLK], F32, tag=f"t2{cc}")
                if last_ib:
                    nc.scalar.activation(t2, t, AF.Identity,
                                         bias=g3_sb[:, cc:cc + 1],
                                         scale=gamma_sb[:, 0:1])
                else:
                    nc.gpsimd.tensor_scalar(t2, t, b3_sb[:, cc:cc + 1],
                                            gamma_sb, OP.add, OP.mult)
                eng = nc.vector if last_ib else nc.gpsimd
                eng.tensor_tensor(ot[:, cc, :], t2,
                                  x_sb[:, cc, ts(ib, IBLK)], OP.add)
            # single output DMA per i-block (fewer descriptors + fewer
            # ~640ns Sync issue slots)
            nc.sync.dma_start(outr[:, :, ts(ib, IBLK)], ot)

        def issue_en(ib, jc, enp, ens):
            ep = enp.tile([P, IBLK], F32, tag="en")
            nc.tensor.matmul(ep, ksb[:, ts(jc, P)], qsb[:, ts(ib, IBLK)],
                             start=True, stop=True)
            et = ens.tile([P, IBLK], F32R, tag="et")
            nc.scalar.activation(et, ep, AF.Exp)
            return et

        with tc.tile_pool(name="ens", bufs=6) as ens, \
             tc.tile_pool(name="fin", bufs=2) as fin:
            # ---- phase 1: projections fused with attention i-block 0 ---
            # x takes ~12 us to land (4 MiB at ~358 GB/s); interleaving
            # i-block 0's attention groups (which consume only already-
            # projected k/v chunks and q columns 0:512) fills that wait.
            # Emitted per 512-col block b: [q(b) k(b) v(4b..4b+3)] then the
            # attention group for the PREVIOUS block's j-chunks.
            with tc.tile_pool(name="pqk_ps", bufs=1, space="PSUM") as pqk, \
                 tc.tile_pool(name="pv_ps0", bufs=2, space="PSUM") as ppv, \
                 tc.tile_pool(name="en_ps0", bufs=2, space="PSUM") as enp0, \
                 tc.tile_pool(name="acc_ps0", bufs=1, space="PSUM") as acc0:

                def proj_block(b):
                    # x chunk b's f32r conversion is emitted here so the
                    # Vector queue stays in true dependency order (no
                    # head-of-line blocking behind later chunks)
                    if b > 0:
                        conv_chunk(b)
                    if b < HALF // PB:
                        # q = w1 @ x[:, 0:HALF] + b1  (rows 32+ zero)
                        qp = pqk.tile([P, PB], F32, tag="qk")
                        nc.tensor.matmul(qp, w1t_r[:, 0, :],
                                         x_r[:, 0, ts(b, PB)],
                                         start=True, stop=False)
                        nc.tensor.matmul(qp, w1t_r[:, 1, :],
                                         x_r[:, 1, ts(b, PB)],
                                         start=False, stop=True)
                        nc.scalar.activation(qsb[:, ts(b, PB)], qp,
                                             AF.Identity, bias=b1_sb[:, 0:1],
                                             scale=1.0)
                    # k = w2 @ x + b2  (rows 32+ zero)
                    kp = pqk.tile([P, PB], F32, tag="qk")
                    nc.tensor.matmul(kp, w2t_r[:, 0, :], x_r[:, 0, ts(b, PB)],
                                     start=True, stop=False)
                    nc.tensor.matmul(kp, w2t_r[:, 1, :], x_r[:, 1, ts(b, PB)],
                                     start=False, stop=True)
                    nc.scalar.activation(ksb[:, ts(b, PB)], kp, AF.Identity,
                                         bias=b2_sb[:, 0:1], scale=1.0)
                    # vT[j, c] = sum_c' x[c', j] * w3t[c', c]   (no bias)
                    for jc in range(4 * b, 4 * b + 4):
                        vp = ppv.tile([P, C], F32, tag="v")
                        nc.tensor.matmul(vp, x_r[:, 0, ts(jc, P)],
                                         w3t_r[:, 0, :], start=True,
                                         stop=False)
                        nc.tensor.matmul(vp, x_r[:, 1, ts(jc, P)],
                                         w3t_r[:, 1, :], start=False,
                                         stop=True)
                        nc.vector.tensor_copy(vts[:, jc, :], vp)

                pv0 = acc0.tile([P, IBLK], F32, tag="pv0")
                pv1 = acc0.tile([P, IBLK], F32, tag="pv1")
                sp = acc0.tile([P, IBLK], F32, tag="s")
                proj_block(0)
                ets = {0: issue_en(0, 0, enp0, ens)}
                for jc in range(NJ):
                    first, last = jc == 0, jc == NJ - 1
                    et = ets.pop(jc)
                    nc.tensor.matmul(sp, ones_sb, et, start=first, stop=last,
                                     skip_group_check=True)
                    nc.tensor.matmul(pv0, vts[:, jc, 0:P], et, start=first,
                                     stop=last, skip_group_check=True)
                    nc.tensor.matmul(pv1, vts[:, jc, P:C], et, start=first,
                                     stop=last, skip_group_check=True)
                    if jc + 1 < NJ:
                        ets[jc + 1] = issue_en(0, jc + 1, enp0, ens)
                    # next block's projections after this block's first
                    # attention group (exp(jc+1) precedes them in the
                    # Scalar queue, so EXP is never head-of-line blocked)
                    if jc % 4 == 0 and jc // 4 + 1 < NXB:
                        proj_block(jc // 4 + 1)
                finalize(0, pv0, pv1, sp, fin, last_ib=False)

            # ---- phase 2: attention i-blocks 1..3 -----------------------
            # Tensor-queue order per i-block:
            #   en(0)..en(2) | [acc(0) en(3)] [acc(1) en(4)] ... [acc(31)]
            # so exp(jc) (scalar engine) has ~3 matmul groups of slack.
            # en banks first (PSUM banks 0-3), accumulators after; s is
            # double-buffered so the next i-block's sum accumulation is not
            # gated on the reciprocal of the previous one.
            LOOKAHEAD = 3
            with tc.tile_pool(name="en_ps", bufs=4, space="PSUM") as enp, \
                 tc.tile_pool(name="pv_ps", bufs=1, space="PSUM") as pvp, \
                 tc.tile_pool(name="s_ps", bufs=2, space="PSUM") as ssp:
                for ib in range(1, NIB):
                    pv0 = pvp.tile([P, IBLK], F32, tag="pv0")
                    pv1 = pvp.tile([P, IBLK], F32, tag="pv1")
                    sp = ssp.tile([P, IBLK], F32, tag="s")
                    ets = {}
                    for jc in range(LOOKAHEAD):
                        ets[jc] = issue_en(ib, jc, enp, ens)
                    for jc in range(NJ):
                        first, last = jc == 0, jc == NJ - 1
                        et = ets.pop(jc)
                        nc.tensor.matmul(sp, ones_sb, et, start=first,
                                         stop=last, skip_group_check=True)
                        nc.tensor.matmul(pv0, vts[:, jc, 0:P], et,
                                         start=first, stop=last,
                                         skip_group_check=True)
                        nc.tensor.matmul(pv1, vts[:, jc, P:C], et,
                                         start=first, stop=last,
                                         skip_group_check=True)
                        if jc + LOOKAHEAD < NJ:
                            ets[jc + LOOKAHEAD] = issue_en(ib, jc + LOOKAHEAD,
                                                           enp, ens)
                    finalize(ib, pv0, pv1, sp, fin, last_ib=ib == NIB - 1)


def kernel(x, w1, b1, w2, b2, w3, b3, gamma, trace=False):
    global last_results
    x = np.ascontiguousarray(np.asarray(x, dtype=np.float32))
    w1t = np.ascontiguousarray(np.asarray(w1, np.float32).T)
    w2t = np.ascontiguousarray(np.asarray(w2, np.float32).T)
    w3t = np.ascontiguousarray(np.asarray(w3, np.float32).T)
    b1 = np.ascontiguousarray(np.asarray(b1, np.float32))
    b2 = np.ascontiguousarray(np.asarray(b2, np.float32))
    b3 = np.ascontiguousarray(np.asarray(b3, np.float32))
    gamma = np.ascontiguousarray(np.asarray(gamma, np.float32))

    if "nc" not in _cache:
        _cache["nc"] = _build_nc()
    nc = _cache["nc"]

    xf = x.reshape(B, C, N)
    in_maps = []
    for core in range(NCORES):
        b, h = divmod(core, 2)
        # rotate so this core's query rows are local columns 0:HALF
        # (attention is permutation-invariant over keys/values)
        xb = xf[b]
        if h:
            xb = np.concatenate([xb[:, HALF:], xb[:, :HALF]], axis=1)
        in_maps.append({
            "x": np.ascontiguousarray(xb),
            "w1t": w1t, "w2t": w2t, "w3t": w3t,
            "b1": b1, "b2": b2, "b3": b3, "gamma": gamma,
        })

    res = bass_utils.run_bass_kernel_spmd(
        nc, in_maps, core_ids=list(range(NCORES)), trace=trace)
    last_results = res

    out = np.empty((B, C, N), np.float32)
    for core in range(NCORES):
        b, h = divmod(core, 2)
        out[b][:, h * HALF:(h + 1) * HALF] = res.results[core]["out"]
    return out.reshape(B, C, H, W)

